# revision 21
# baseline (speedup 1.0000x reference)
"""Trainium2 Bass kernel for nn_ComplexMixture (weighted complex density
matrices).

Reference computation (B=4, S=8192, D=512):
    out_r[b] = sum_s w[b,s] * (r_s r_s^T + i_s i_s^T)   -> [B, D, D]
    out_i[b] = sum_s w[b,s] * (i_s r_s^T - r_s i_s^T)   -> [B, D, D]

Strategy (8 NeuronCores):
  - Shard (b, S-half): core k handles batch k//2, S rows [4096*(k%2), +4096).
  - Fold sqrt(w) into both operands (w >= 0): Rs = sqrt(w)*R, Is = sqrt(w)*I.
        out_r = Rs^T Rs + Is^T Is   (symmetric -> block-upper only)
        out_i = G - G^T,  G = Is^T Rs  (host antisymmetrizes)
  - G runs in bf16 (full PE rate, fp32 PSUM accumulate). The two symmetric
    grams run in fp8(e4m3) with MatmulPerfMode.DoubleRow: one instruction
    contracts a PAIR of 128-row subtiles at 0.5 cycles/output-column.
    PE cost/subtile: G 2048 + grams 640 = ~1.12us, ~30% slack vs the
    ~1.6us/subtile DMA delivery rate, so the PE absorbs stream jitter.
  - DMA layout: partition p holds DRAM rows [32p, 32p+32) -> 64 KiB
    contiguous per partition; chunk loads use 2-8 KiB descriptors on two
    HWDGE rings (sync=xr, scalar=xi). Subtile j for the matmul contraction
    is the strided view plane[:, j*512:(j+1)*512].
  - 8 PSUM banks: G m0..m3 full banks; grams (Rs+Is accumulate together)
    one bank per m. Tail: bf16 PSUM->SBUF copies packed into two staging
    tiles, two DMAs total; host unpacks/combines/mirrors.
  - Accuracy: fp8 grams -> out_r ~8e-3, bf16 G -> out_i ~3e-3 rel err
    (gate 2e-2).
"""

import sys

if "/opt/trn_rl_repo" not in sys.path:
    sys.path.insert(0, "/opt/trn_rl_repo")

import numpy as np

B, S, D = 4, 8192, 512
N_CORES = 8
S_LOC = S // 2          # rows per core
P = 128                 # SBUF partitions
J = S_LOC // P          # 32 subtiles per core
# upper-triangle column starts per m-chunk
C0 = (0, 128, 256, 384)
# DMA chunk sizes in subtiles (sized so delivery tracks PE consumption)
CHUNKS = (1, 1, 2, 2, 2, 4, 4, 4, 4, 4, 4)
# packed gram output column offsets per m (upper widths 512/384/256/128)
OFF = (0, 512, 896, 1152)
WTOT = 1280

_cache = {}


def _split_multi_waits(bir: bytes) -> bytes:
    """This container's walrus build accepts at most one sync-wait command
    per instruction ("Too many sync wait commands"), while Tile freely packs
    several. Splitting the extras into preceding single-wait NoOps on the
    same engine is semantically identical for monotonic sem-ge waits: the
    sequencer blocks on each in turn before dispatching the instruction.
    """
    import json

    m = json.loads(bir)
    n = [0]

    def fix(obj):
        if isinstance(obj, dict):
            insts = obj.get("instructions")
            if isinstance(insts, list) and insts and isinstance(insts[0], dict):
                out = []
                for inst in insts:
                    si = inst.get("sync_info")
                    waits = (si or {}).get("on_wait") or []
                    cap = 2 if inst.get("opcode") == "EventSemaphore" else 1
                    if len(waits) > cap and all(
                        w.get("wait_mode") == "sem-ge-imm" for w in waits
                    ):
                        for w in waits[:-cap]:
                            n[0] += 1
                            nop = {
                                "engine": inst["engine"],
                                "ins": [],
                                "name": f"{inst['name']}-ws{n[0]}",
                                "opcode": "NoOp",
                                "outs": [],
                                "sync_info": {"on_wait": [w], "on_update": []},
                                "text_hint": "wait_split",
                            }
                            if "debug" in inst:
                                nop["debug"] = inst["debug"]
                            out.append(nop)
                        si["on_wait"] = waits[-cap:]
                    out.append(inst)
                obj["instructions"] = out
            for v in obj.values():
                fix(v)
        elif isinstance(obj, list):
            for v in obj:
                fix(v)

    fix(m)
    return json.dumps(m).encode()


def _install_wait_split_patch(bass):
    if getattr(bass.Bass, "_wait_split_patched", False):
        return
    orig = bass.Bass.to_json_bytes

    def to_json_bytes(self, *a, **kw):
        return _split_multi_waits(orig(self, *a, **kw))

    bass.Bass.to_json_bytes = to_json_bytes
    bass.Bass._wait_split_patched = True


def _build():
    import concourse.bass as bass
    import concourse.tile as tile
    from concourse import mybir

    _install_wait_split_patch(bass)
    f32 = mybir.dt.float32
    bf16 = mybir.dt.bfloat16
    fp8 = mybir.dt.float8e4
    DR = mybir.MatmulPerfMode.DoubleRow

    nc = bass.Bass()
    xr = nc.dram_tensor("xr", [S_LOC, D], f32, kind="ExternalInput")
    xi = nc.dram_tensor("xi", [S_LOC, D], f32, kind="ExternalInput")
    ws = nc.dram_tensor("ws", [P, J], f32, kind="ExternalInput")
    # packed outputs, bf16: G full (m-major) and gram upper blocks
    o_g = nc.dram_tensor("o_g", [P, 4 * D], bf16, kind="ExternalOutput")
    o_r = nc.dram_tensor("o_r", [P, WTOT], bf16, kind="ExternalOutput")

    # partition p <- rows [32p, 32p+32): 64 KiB contiguous per partition
    xr4 = xr.rearrange("(p j) d -> p (j d)", p=P)
    xi4 = xi.rearrange("(p j) d -> p (j d)", p=P)

    with tile.TileContext(nc) as tc:
        with (
            tc.tile_pool(name="big", bufs=1) as big,
            tc.tile_pool(name="wp", bufs=1) as wp,
            tc.tile_pool(name="raw", bufs=2) as raw,
            tc.tile_pool(name="psum", bufs=1, space="PSUM") as psum,
            tc.tile_pool(name="ost", bufs=1) as ost,
        ):
            rs = big.tile([P, J * D], bf16, name="rs", tag="rs")
            im = big.tile([P, J * D], bf16, name="im", tag="im")
            r8 = big.tile([P, J, D], fp8, name="r8", tag="r8")
            i8 = big.tile([P, J, D], fp8, name="i8", tag="i8")
            wt = wp.tile([P, J], f32, name="wt", tag="wt")
            dmy = wp.tile([P, P], f32, name="dmy", tag="dmy")

            nc.sync.dma_start(wt[:], ws[:])
            # Preload the ACT Copy table during the DMA lead-in.
            nc.vector.memset(dmy[:], 0.0)
            nc.scalar.mul(dmy[:, :1], dmy[:, :1], 1.0)

            # 8 PSUM banks: G m0..3 full; grams one bank per m (Rs- and
            # Is-gram accumulate into the same group -> out_r directly).
            gb = [psum.tile([P, D], f32, name=f"g{m}", tag=f"g{m}") for m in range(4)]
            W = [D - C0[m] for m in range(4)]
            rb = [
                psum.tile([P, W[m]], f32, name=f"r{m}", tag=f"r{m}")
                for m in range(4)
            ]

            # PE warm-up during the DMA lead-in (HAM un-throttles after
            # sustained activity). fp32 dummies into g0; the first real
            # start=True matmul there discards them.
            for _ in range(4):
                nc.tensor.matmul(
                    gb[0][:, :P], dmy[:], dmy[:], start=True, stop=True,
                    skip_group_check=True,
                )

            # ---- streaming: DMA chunks -> scale/convert -> matmuls ------
            j0 = 0
            for ci, ch in enumerate(CHUNKS):
                w_ch = ch * D
                a = raw.tile([P, w_ch], f32, name=f"rawr{ci}", tag="rawr")
                nc.sync.dma_start(a[:], xr4[:, j0 * D : j0 * D + w_ch])
                c = raw.tile([P, w_ch], f32, name=f"rawi{ci}", tag="rawi")
                nc.scalar.dma_start(c[:], xi4[:, j0 * D : j0 * D + w_ch])
                for q in range(ch):
                    j = j0 + q
                    sl = slice(j * D, (j + 1) * D)
                    si = slice(q * D, (q + 1) * D)
                    wj = wt[:, j : j + 1]
                    nc.scalar.mul(im[:, sl], c[:, si], wj)
                    nc.vector.tensor_scalar_mul(rs[:, sl], a[:, si], wj)
                    nc.vector.tensor_copy(r8[:, j, :], rs[:, sl])
                    nc.vector.tensor_copy(i8[:, j, :], im[:, sl])
                    # G matmuls for subtile j (bf16, full rows)
                    for m in range(4):
                        nc.tensor.matmul(
                            gb[m][:],
                            im[:, j * D + m * P : j * D + (m + 1) * P],
                            rs[:, sl],
                            start=(j == 0), stop=(j == J - 1),
                        )
                    if j % 2 == 1:
                        # gram DoubleRow matmuls for the pair (j-1, j)
                        pst, psp = (j == 1), (j == J - 1)
                        for m in range(4):
                            c0 = C0[m]
                            nc.tensor.matmul(
                                rb[m][:],
                                r8[:, j - 1 : j + 1, m * P : (m + 1) * P],
                                r8[:, j - 1 : j + 1, c0:D],
                                start=pst, stop=False, perf_mode=DR,
                            )
                            nc.tensor.matmul(
                                rb[m][:],
                                i8[:, j - 1 : j + 1, m * P : (m + 1) * P],
                                i8[:, j - 1 : j + 1, c0:D],
                                start=False, stop=psp, perf_mode=DR,
                            )
                j0 += ch

            # ---- drain: bf16 PSUM->SBUF copies into packed staging, then
            # one DMA per output; host unpacks/combines ----
            sg = ost.tile([P, 4 * D], bf16, name="sg", tag="sg")
            sr = ost.tile([P, WTOT], bf16, name="sr", tag="sr")
            for m in range(4):
                o = slice(OFF[m], OFF[m] + W[m])
                g = slice(m * D, (m + 1) * D)
                if m < 2:
                    nc.vector.tensor_copy(sg[:, g], gb[m][:])
                    nc.scalar.copy(sr[:, o], rb[m][:])
                else:
                    nc.scalar.copy(sg[:, g], gb[m][:])
                    nc.vector.tensor_copy(sr[:, o], rb[m][:])
            nc.scalar.dma_start(o_g[:], sg[:])
            nc.sync.dma_start(o_r[:], sr[:])

    return nc


def _get_nc():
    if "nc" not in _cache:
        _cache["nc"] = _build()
    return _cache["nc"]


def kernel(input_real, input_imag, weight):
    from concourse.bass_utils import run_bass_kernel_spmd

    input_real = np.ascontiguousarray(input_real, dtype=np.float32)
    input_imag = np.ascontiguousarray(input_imag, dtype=np.float32)
    weight = np.asarray(weight, dtype=np.float32)
    sw = np.sqrt(weight)  # w >= 0 (uniform fill)

    in_maps = []
    for k in range(N_CORES):
        b, h = k // 2, k % 2
        rows = slice(h * S_LOC, (h + 1) * S_LOC)
        in_maps.append(
            {
                "xr": np.ascontiguousarray(input_real[b, rows, :]),
                "xi": np.ascontiguousarray(input_imag[b, rows, :]),
                # ws[p, j] = sqrt(w[b, h*S_LOC + 32p + j])
                "ws": np.ascontiguousarray(sw[b, rows].reshape(P, J)),
            }
        )

    res = run_bass_kernel_spmd(
        _get_nc(), in_maps, core_ids=list(range(N_CORES))
    )

    def unpack_g(packed):
        # packed [128, 4*512] (m-major) -> G [512, 512]
        return packed.astype(np.float32).reshape(P, 4, D).transpose(1, 0, 2).reshape(D, D)

    def unpack_r(packed):
        # packed [128, 1280] -> [512, 512] upper blocks at natural positions
        full = np.zeros((D, D), dtype=np.float32)
        for m in range(4):
            w = D - C0[m]
            full[m * P : (m + 1) * P, C0[m] :] = packed[
                :, OFF[m] : OFF[m] + w
            ].astype(np.float32)
        return full

    out_r = np.empty((B, D, D), dtype=np.float32)
    out_i = np.empty((B, D, D), dtype=np.float32)
    for b in range(B):
        r0, r1 = res.results[2 * b], res.results[2 * b + 1]
        G = unpack_g(r0["o_g"]) + unpack_g(r1["o_g"])
        Ru = unpack_r(r0["o_r"]) + unpack_r(r1["o_r"])
        out_i[b] = G - G.T
        F = np.empty((D, D), dtype=np.float32)
        for m in range(4):
            rm = slice(m * P, (m + 1) * P)
            for n in range(4):
                rn = slice(n * P, (n + 1) * P)
                if m <= n:
                    F[rm, rn] = Ru[rm, rn]
                else:
                    F[rm, rn] = Ru[rn, rm].T
        out_r[b] = F
    return out_r, out_i


# revision 22
# speedup vs baseline: 1.0724x; 1.0724x over previous
"""Trainium2 Bass kernel for nn_ComplexMixture (weighted complex density
matrices).

Reference computation (B=4, S=8192, D=512):
    out_r[b] = sum_s w[b,s] * (r_s r_s^T + i_s i_s^T)   -> [B, D, D]
    out_i[b] = sum_s w[b,s] * (i_s r_s^T - r_s i_s^T)   -> [B, D, D]

Strategy (8 NeuronCores):
  - Shard (b, S-half): core k handles batch k//2, S rows [4096*(k%2), +4096).
  - Fold sqrt(w) into both operands (w >= 0): Rs = sqrt(w)*R, Is = sqrt(w)*I.
        out_r = Rs^T Rs + Is^T Is   (symmetric -> block-upper only)
        out_i = G - G^T,  G = Is^T Rs  (host antisymmetrizes)
  - G runs in bf16 (full PE rate, fp32 PSUM accumulate). The two symmetric
    grams run in fp8(e4m3) with MatmulPerfMode.DoubleRow: one instruction
    contracts a PAIR of 128-row subtiles at 0.5 cycles/output-column.
    PE cost/subtile: G 2048 + grams 640 = ~1.12us, ~30% slack vs the
    ~1.6us/subtile DMA delivery rate, so the PE absorbs stream jitter.
  - DMA layout: partition p holds DRAM rows [32p, 32p+32) -> 64 KiB
    contiguous per partition; chunk loads use 2-8 KiB descriptors on two
    HWDGE rings (sync=xr, scalar=xi). Subtile j for the matmul contraction
    is the strided view plane[:, j*512:(j+1)*512].
  - 8 PSUM banks: G m0..m3 full banks; grams (Rs+Is accumulate together)
    one bank per m. Tail: bf16 PSUM->SBUF copies packed into two staging
    tiles, two DMAs total; host unpacks/combines/mirrors.
  - Accuracy: fp8 grams -> out_r ~8e-3, bf16 G -> out_i ~3e-3 rel err
    (gate 2e-2).
"""

import sys

if "/opt/trn_rl_repo" not in sys.path:
    sys.path.insert(0, "/opt/trn_rl_repo")

import numpy as np

B, S, D = 4, 8192, 512
N_CORES = 8
S_LOC = S // 2          # rows per core
P = 128                 # SBUF partitions
J = S_LOC // P          # 32 subtiles per core
# upper-triangle column starts per m-chunk
C0 = (0, 128, 256, 384)
# DMA chunk sizes in subtiles (sized so delivery tracks PE consumption)
CHUNKS = (1, 1, 2, 2, 2, 4, 4, 4, 4, 4, 4)
# packed gram output column offsets per m (upper widths 512/384/256/128)
OFF = (0, 512, 896, 1152)
WTOT = 1280

_cache = {}


def _split_multi_waits(bir: bytes) -> bytes:
    """This container's walrus build accepts at most one sync-wait command
    per instruction ("Too many sync wait commands"), while Tile freely packs
    several. Splitting the extras into preceding single-wait NoOps on the
    same engine is semantically identical for monotonic sem-ge waits: the
    sequencer blocks on each in turn before dispatching the instruction.
    """
    import json

    m = json.loads(bir)
    n = [0]

    def fix(obj):
        if isinstance(obj, dict):
            insts = obj.get("instructions")
            if isinstance(insts, list) and insts and isinstance(insts[0], dict):
                out = []
                for inst in insts:
                    si = inst.get("sync_info")
                    waits = (si or {}).get("on_wait") or []
                    cap = 2 if inst.get("opcode") == "EventSemaphore" else 1
                    if len(waits) > cap and all(
                        w.get("wait_mode") == "sem-ge-imm" for w in waits
                    ):
                        for w in waits[:-cap]:
                            n[0] += 1
                            nop = {
                                "engine": inst["engine"],
                                "ins": [],
                                "name": f"{inst['name']}-ws{n[0]}",
                                "opcode": "NoOp",
                                "outs": [],
                                "sync_info": {"on_wait": [w], "on_update": []},
                                "text_hint": "wait_split",
                            }
                            if "debug" in inst:
                                nop["debug"] = inst["debug"]
                            out.append(nop)
                        si["on_wait"] = waits[-cap:]
                    out.append(inst)
                obj["instructions"] = out
            for v in obj.values():
                fix(v)
        elif isinstance(obj, list):
            for v in obj:
                fix(v)

    fix(m)
    return json.dumps(m).encode()


def _install_wait_split_patch(bass):
    if getattr(bass.Bass, "_wait_split_patched", False):
        return
    orig = bass.Bass.to_json_bytes

    def to_json_bytes(self, *a, **kw):
        return _split_multi_waits(orig(self, *a, **kw))

    bass.Bass.to_json_bytes = to_json_bytes
    bass.Bass._wait_split_patched = True


def _build():
    import concourse.bass as bass
    import concourse.tile as tile
    from concourse import mybir

    _install_wait_split_patch(bass)
    f32 = mybir.dt.float32
    bf16 = mybir.dt.bfloat16
    fp8 = mybir.dt.float8e4
    DR = mybir.MatmulPerfMode.DoubleRow

    nc = bass.Bass()
    xr = nc.dram_tensor("xr", [S_LOC, D], f32, kind="ExternalInput")
    xi = nc.dram_tensor("xi", [S_LOC, D], f32, kind="ExternalInput")
    ws = nc.dram_tensor("ws", [P, J], f32, kind="ExternalInput")
    # packed outputs, bf16: G full (m-major) and gram upper blocks
    o_g = nc.dram_tensor("o_g", [P, 4 * D], bf16, kind="ExternalOutput")
    o_r = nc.dram_tensor("o_r", [P, WTOT], bf16, kind="ExternalOutput")

    # partition p <- rows [32p, 32p+32): 64 KiB contiguous per partition
    xr4 = xr.rearrange("(p j) d -> p (j d)", p=P)
    xi4 = xi.rearrange("(p j) d -> p (j d)", p=P)

    with tile.TileContext(nc) as tc:
        with (
            tc.tile_pool(name="big", bufs=1) as big,
            tc.tile_pool(name="wp", bufs=1) as wp,
            tc.tile_pool(name="raw", bufs=2) as raw,
            tc.tile_pool(name="psum", bufs=1, space="PSUM") as psum,
            tc.tile_pool(name="ost", bufs=1) as ost,
        ):
            rs = big.tile([P, J * D], bf16, name="rs", tag="rs")
            im = big.tile([P, J * D], bf16, name="im", tag="im")
            r8 = big.tile([P, J, D], fp8, name="r8", tag="r8")
            i8 = big.tile([P, J, D], fp8, name="i8", tag="i8")
            wt = wp.tile([P, J], f32, name="wt", tag="wt")
            dmy = wp.tile([P, P], f32, name="dmy", tag="dmy")

            nc.sync.dma_start(wt[:], ws[:])
            # Preload the ACT Copy table during the DMA lead-in.
            nc.vector.memset(dmy[:], 0.0)
            nc.scalar.mul(dmy[:, :1], dmy[:, :1], 1.0)

            # 8 PSUM banks: G m0..3 full; grams one bank per m (Rs- and
            # Is-gram accumulate into the same group -> out_r directly).
            gb = [psum.tile([P, D], f32, name=f"g{m}", tag=f"g{m}") for m in range(4)]
            W = [D - C0[m] for m in range(4)]
            rb = [
                psum.tile([P, W[m]], f32, name=f"r{m}", tag=f"r{m}")
                for m in range(4)
            ]

            # PE warm-up during the DMA lead-in (HAM un-throttles after
            # sustained activity). fp32 dummies into g0; the first real
            # start=True matmul there discards them.
            for _ in range(4):
                nc.tensor.matmul(
                    gb[0][:, :P], dmy[:], dmy[:], start=True, stop=True,
                    skip_group_check=True,
                )

            # ---- streaming: DMA chunks -> scale/convert -> matmuls ------
            j0 = 0
            for ci, ch in enumerate(CHUNKS):
                w_ch = ch * D
                a = raw.tile([P, w_ch], f32, name=f"rawr{ci}", tag="rawr")
                nc.sync.dma_start(a[:], xr4[:, j0 * D : j0 * D + w_ch])
                c = raw.tile([P, w_ch], f32, name=f"rawi{ci}", tag="rawi")
                nc.scalar.dma_start(c[:], xi4[:, j0 * D : j0 * D + w_ch])
                for q in range(ch):
                    j = j0 + q
                    sl = slice(j * D, (j + 1) * D)
                    si = slice(q * D, (q + 1) * D)
                    wj = wt[:, j : j + 1]
                    nc.scalar.mul(im[:, sl], c[:, si], wj)
                    nc.vector.tensor_scalar_mul(rs[:, sl], a[:, si], wj)
                    if j % 2 == 1:
                        # fp8 converts amortized over subtile pairs
                        pr = slice((j - 1) * D, (j + 1) * D)
                        nc.vector.tensor_copy(r8[:, j - 1 : j + 1, :], rs[:, pr])
                        nc.scalar.copy(i8[:, j - 1 : j + 1, :], im[:, pr])
                    # G matmuls for subtile j (bf16, full rows)
                    for m in range(4):
                        nc.tensor.matmul(
                            gb[m][:],
                            im[:, j * D + m * P : j * D + (m + 1) * P],
                            rs[:, sl],
                            start=(j == 0), stop=(j == J - 1),
                        )
                    if j % 2 == 1:
                        # gram DoubleRow matmuls for the pair (j-1, j)
                        pst, psp = (j == 1), (j == J - 1)
                        for m in range(4):
                            c0 = C0[m]
                            nc.tensor.matmul(
                                rb[m][:],
                                r8[:, j - 1 : j + 1, m * P : (m + 1) * P],
                                r8[:, j - 1 : j + 1, c0:D],
                                start=pst, stop=False, perf_mode=DR,
                            )
                            nc.tensor.matmul(
                                rb[m][:],
                                i8[:, j - 1 : j + 1, m * P : (m + 1) * P],
                                i8[:, j - 1 : j + 1, c0:D],
                                start=False, stop=psp, perf_mode=DR,
                            )
                j0 += ch

            # ---- drain: bf16 PSUM->SBUF copies into packed staging, then
            # one DMA per output; host unpacks/combines ----
            sg = ost.tile([P, 4 * D], bf16, name="sg", tag="sg")
            sr = ost.tile([P, WTOT], bf16, name="sr", tag="sr")
            for m in range(4):
                o = slice(OFF[m], OFF[m] + W[m])
                g = slice(m * D, (m + 1) * D)
                if m < 2:
                    nc.vector.tensor_copy(sg[:, g], gb[m][:])
                    nc.scalar.copy(sr[:, o], rb[m][:])
                else:
                    nc.scalar.copy(sg[:, g], gb[m][:])
                    nc.vector.tensor_copy(sr[:, o], rb[m][:])
            nc.scalar.dma_start(o_g[:], sg[:])
            nc.sync.dma_start(o_r[:], sr[:])

    return nc


def _get_nc():
    if "nc" not in _cache:
        _cache["nc"] = _build()
    return _cache["nc"]


def kernel(input_real, input_imag, weight):
    from concourse.bass_utils import run_bass_kernel_spmd

    input_real = np.ascontiguousarray(input_real, dtype=np.float32)
    input_imag = np.ascontiguousarray(input_imag, dtype=np.float32)
    weight = np.asarray(weight, dtype=np.float32)
    sw = np.sqrt(weight)  # w >= 0 (uniform fill)

    in_maps = []
    for k in range(N_CORES):
        b, h = k // 2, k % 2
        rows = slice(h * S_LOC, (h + 1) * S_LOC)
        in_maps.append(
            {
                "xr": np.ascontiguousarray(input_real[b, rows, :]),
                "xi": np.ascontiguousarray(input_imag[b, rows, :]),
                # ws[p, j] = sqrt(w[b, h*S_LOC + 32p + j])
                "ws": np.ascontiguousarray(sw[b, rows].reshape(P, J)),
            }
        )

    res = run_bass_kernel_spmd(
        _get_nc(), in_maps, core_ids=list(range(N_CORES))
    )

    def unpack_g(packed):
        # packed [128, 4*512] (m-major) -> G [512, 512]
        return packed.astype(np.float32).reshape(P, 4, D).transpose(1, 0, 2).reshape(D, D)

    def unpack_r(packed):
        # packed [128, 1280] -> [512, 512] upper blocks at natural positions
        full = np.zeros((D, D), dtype=np.float32)
        for m in range(4):
            w = D - C0[m]
            full[m * P : (m + 1) * P, C0[m] :] = packed[
                :, OFF[m] : OFF[m] + w
            ].astype(np.float32)
        return full

    out_r = np.empty((B, D, D), dtype=np.float32)
    out_i = np.empty((B, D, D), dtype=np.float32)
    for b in range(B):
        r0, r1 = res.results[2 * b], res.results[2 * b + 1]
        G = unpack_g(r0["o_g"]) + unpack_g(r1["o_g"])
        Ru = unpack_r(r0["o_r"]) + unpack_r(r1["o_r"])
        out_i[b] = G - G.T
        F = np.empty((D, D), dtype=np.float32)
        for m in range(4):
            rm = slice(m * P, (m + 1) * P)
            for n in range(4):
                rn = slice(n * P, (n + 1) * P)
                if m <= n:
                    F[rm, rn] = Ru[rm, rn]
                else:
                    F[rm, rn] = Ru[rn, rm].T
        out_r[b] = F
    return out_r, out_i
